# revision 5
# baseline (speedup 1.0000x reference)
"""DCRNN (K=1, H0=0) fused kernel for 8 Trainium2 NeuronCores.

Math (derived from the reference with H0 = 0):
    R is dead (multiplied by H0=0); XH == XHR == [x, 0].
    Az = (Wz[0] + Wz[1])[:F]           # [256, 32]
    Ah = (Wh[0] + Wh[1])[:F]           # [256, 32]
    Zc = sigmoid(-(x @ Az + bz))       # == 1 - Z, strictly positive
    T  = tanh(x @ Ah + bh)
    h  = relu(Zc * T) == Zc * relu(T)
    y  = h @ Wl + bl                   # [N, 1]

Strategy: data-parallel over nodes on 8 cores.  Per core, x-shard rows are
DMA-transpose-loaded (bf16) so features sit on partitions; each 128-node
subtile of x.T is the *stationary* matmul operand against the small moving
weight [128, 64] = [Az|Ah] chunk, so pre-activations land in natural
orientation [128 nodes, 64] in PSUM.  Biases are added with a K=1 rank-1
matmul (ones ⊗ biascat).  ScalarE applies sigmoid(-pre)/tanh straight out
of PSUM; VectorE fuses relu+mult, applies Wl and row-reduces to y.  y is
PE-transposed at the end so the store is one contiguous DMA.
"""

import sys

import numpy as np

sys.path.insert(0, "/opt/trn_rl_repo")

import ml_dtypes

N = 200000
F = 256
HID = 32
NCORES = 8
PER = 25088            # padded nodes per core: 25088 = 24*1024 + 512
NPAD = PER * NCORES    # 200704
SUPER = 1024           # nodes per superblock (8 subtiles of 128)
NSUPER = 25            # 24 full superblocks + 1 half (512 nodes)
YCOLS = PER // 128     # 196

BF16 = ml_dtypes.bfloat16

_PROGS = {}


def _build_program(reps=1):
    import contextlib

    import concourse.tile as tile
    from concourse import bacc, mybir

    BF = mybir.dt.bfloat16
    F32 = mybir.dt.float32
    AF = mybir.ActivationFunctionType
    OP = mybir.AluOpType

    nc = bacc.Bacc("TRN2", target_bir_lowering=False, debug=False,
                   num_devices=NCORES)

    x_d = nc.dram_tensor("x", [PER, F], BF, kind="ExternalInput").ap()
    acat_d = nc.dram_tensor("acat", [2, 128, 64], BF, kind="ExternalInput").ap()
    bias_d = nc.dram_tensor("biascat", [1, 512], BF, kind="ExternalInput").ap()
    wl_d = nc.dram_tensor("wlfull", [128, 256], BF, kind="ExternalInput").ap()
    ones_d = nc.dram_tensor("ones", [1, 128], BF, kind="ExternalInput").ap()
    id_d = nc.dram_tensor("ident", [128, 128], F32, kind="ExternalInput").ap()
    y_d = nc.dram_tensor("y", [YCOLS, 128], F32, kind="ExternalOutput").ap()

    with tile.TileContext(nc) as tc:
        with tc.tile_pool(name="const", bufs=1) as cp, \
             tc.tile_pool(name="xt", bufs=4) as xp, \
             tc.tile_pool(name="act", bufs=3) as vp, \
             tc.tile_pool(name="ps", bufs=4, space="PSUM") as pp, \
             tc.tile_pool(name="yps", bufs=2, space="PSUM") as yp:

            acat0 = cp.tile([128, 64], BF)
            acat1 = cp.tile([128, 64], BF)
            biascat = cp.tile([1, 512], BF)
            wlfull = cp.tile([128, 256], BF)
            ones = cp.tile([1, 128], BF)
            ident = cp.tile([128, 128], F32)
            ysb = cp.tile([128, YCOLS], F32)

            nc.sync.dma_start(out=acat0[:], in_=acat_d[0])
            nc.sync.dma_start(out=acat1[:], in_=acat_d[1])
            nc.sync.dma_start(out=biascat[:], in_=bias_d[:])
            nc.sync.dma_start(out=wlfull[:], in_=wl_d[:])
            nc.sync.dma_start(out=ones[:], in_=ones_d[:])
            nc.sync.dma_start(out=ident[:], in_=id_d[:])

            rep_ctx = (tc.For_i(0, reps, 1,
                               hint_engines=(mybir.EngineType.PE,
                                             mybir.EngineType.SP))
                       if reps > 1 else contextlib.nullcontext())
            with rep_ctx:
                _kernel_body(nc, tc, mybir, BF, F32, AF, OP,
                             x_d, y_d, xp, vp, pp, yp,
                             acat0, acat1, biascat, wlfull, ones, ident, ysb)

    nc.compile()
    return nc


def _kernel_body(nc, tc, mybir, BF, F32, AF, OP, x_d, y_d, xp, vp, pp, yp,
                 acat0, acat1, biascat, wlfull, ones, ident, ysb):
    if True:
        if True:
            for b in range(NSUPER):
                nsub = 8 if b < NSUPER - 1 else 4
                nn = nsub * 128
                base = b * SUPER

                xt0 = xp.tile([128, SUPER], BF, tag="xt0")
                xt1 = xp.tile([128, SUPER], BF, tag="xt1")
                nc.sync.dma_start(out=xt0[:, :nn],
                                  in_=x_d[base:base + nn, 0:128],
                                  transpose=True)
                nc.sync.dma_start(out=xt1[:, :nn],
                                  in_=x_d[base:base + nn, 128:256],
                                  transpose=True)

                ps = pp.tile([128, 512], F32, tag="ps")
                # rank-1 bias broadcast fills the bank and opens the group
                nc.tensor.matmul(ps[:, :nsub * 64], ones[:],
                                 biascat[:, :nsub * 64],
                                 start=True, stop=False)
                for s in range(nsub):
                    out_sl = ps[:, s * 64:(s + 1) * 64]
                    nc.tensor.matmul(out_sl, xt0[:, s * 128:(s + 1) * 128],
                                     acat0[:], start=False, stop=False)
                    nc.tensor.matmul(out_sl, xt1[:, s * 128:(s + 1) * 128],
                                     acat1[:], start=False,
                                     stop=(s == nsub - 1))

                ps3 = ps[:, :nsub * 64].rearrange("p (s h) -> p s h", h=64)
                zc = vp.tile([128, 256], BF, tag="zc")
                tt = vp.tile([128, 256], BF, tag="tt")
                zc3 = zc[:, :nsub * 32].rearrange("p (s h) -> p s h", h=32)
                tt3 = tt[:, :nsub * 32].rearrange("p (s h) -> p s h", h=32)
                nc.scalar.activation(zc3, ps3[:, :, 0:32], AF.Sigmoid,
                                     scale=-1.0)
                nc.scalar.activation(tt3, ps3[:, :, 32:64], AF.Tanh)

                gr = vp.tile([128, 256], BF, tag="gr")
                gw = vp.tile([128, 256], BF, tag="gw")
                # gr = relu(tt) * zc  (zc > 0 so this equals relu(zc*tt))
                nc.vector.scalar_tensor_tensor(
                    gr[:, :nsub * 32], tt[:, :nsub * 32], 0.0,
                    zc[:, :nsub * 32], op0=OP.max, op1=OP.mult)
                nc.vector.tensor_mul(gw[:, :nsub * 32], gr[:, :nsub * 32],
                                     wlfull[:, :nsub * 32])
                gw3 = gw[:, :nsub * 32].rearrange("p (s h) -> p s h", h=32)
                nc.vector.tensor_reduce(ysb[:, b * 8:b * 8 + nsub], gw3,
                                        axis=mybir.AxisListType.X, op=OP.add)

            half = YCOLS // 2  # 98
            for hh in range(2):
                ytp = yp.tile([half, 128], F32, tag="ytp")
                nc.tensor.transpose(ytp[:], ysb[:, hh * half:(hh + 1) * half],
                                    ident[:])
                yts = vp.tile([half, 128], F32, tag="yts")
                nc.vector.tensor_copy(yts[:], ytp[:])
                nc.sync.dma_start(out=y_d[hh * half:(hh + 1) * half, :],
                                  in_=yts[:])


def _get_program(reps=1):
    if reps not in _PROGS:
        _PROGS[reps] = _build_program(reps)
    return _PROGS[reps]


def _host_inputs(x, Wz, bz, Wh, bh, Wl):
    Az = (np.asarray(Wz[0]) + np.asarray(Wz[1]))[:F]
    Ah = (np.asarray(Wh[0]) + np.asarray(Wh[1]))[:F]
    Acat = np.concatenate([Az, Ah], axis=1)               # [256, 64]
    acat = np.stack([Acat[:128], Acat[128:]]).astype(BF16)
    biascat = np.concatenate([np.asarray(bz), np.asarray(bh)])  # [64]
    biascat8 = np.tile(biascat, 8)[None, :].astype(BF16)  # [1, 512]
    wlfull = np.tile(np.asarray(Wl).reshape(1, HID), (128, 8)).astype(BF16)
    ones = np.ones((1, 128), BF16)
    ident = np.eye(128, dtype=np.float32)

    xb = np.zeros((NPAD, F), dtype=BF16)
    xb[:N] = np.asarray(x).astype(BF16)
    shards = xb.reshape(NCORES, PER, F)
    return shards, acat, biascat8, wlfull, ones, ident


def kernel(x, edge_index, Wz, bz, Wr, br, Wh, bh, Wl, bl, _reps=1):
    from concourse.bass_utils import run_bass_kernel_spmd

    shards, acat, biascat8, wlfull, ones, ident = _host_inputs(
        x, Wz, bz, Wh, bh, Wl)

    nc = _get_program(_reps)
    in_maps = [{
        "x": np.ascontiguousarray(shards[i]),
        "acat": acat,
        "biascat": biascat8,
        "wlfull": wlfull,
        "ones": ones,
        "ident": ident,
    } for i in range(NCORES)]

    res = run_bass_kernel_spmd(nc, in_maps, core_ids=list(range(NCORES)))

    y = np.concatenate([np.asarray(res.results[i]["y"]).reshape(-1)
                        for i in range(NCORES)])[:N]
    out = (y + np.float32(np.asarray(bl).reshape(-1)[0])).astype(np.float32)
    return out.reshape(N, 1)


# revision 33
# speedup vs baseline: 5.5468x; 5.5468x over previous
"""DCRNN (K=1, H0=0) fused kernel for 8 Trainium2 NeuronCores.

Math (derived from the reference with H0 = 0):
    R is dead (multiplied by H0=0); XH == XHR == [x, 0].
    Az = (Wz[0] + Wz[1])[:F]           # [256, 32]
    Ah = (Wh[0] + Wh[1])[:F]           # [256, 32]
    Zc = sigmoid(-(x @ Az + bz))       # == 1 - Z, strictly positive
    T  = tanh(x @ Ah + bh)
    h  = relu(Zc * T) == Zc * relu(T)
    y  = h @ Wl + bl                   # [N, 1]

Strategy: data-parallel over nodes on 8 cores.  Per core, x-shard rows are
DMA-transpose-loaded (bf16) so features sit on partitions; each 128-node
subtile of x.T is the *stationary* matmul operand against the small moving
weight [128, 64] = [Az|Ah] chunk, so pre-activations land in natural
orientation [128 nodes, 64] in PSUM.  Biases are added with a K=1 rank-1
matmul (ones ⊗ biascat).  ScalarE applies sigmoid(-pre)/tanh straight out
of PSUM; VectorE fuses relu+mult, applies Wl and row-reduces to y.  y is
PE-transposed at the end so the store is one contiguous DMA.
"""

import sys

import numpy as np

sys.path.insert(0, "/opt/trn_rl_repo")

import ml_dtypes

N = 200000
F = 256
HID = 32
NCORES = 8
PER = 25088            # padded nodes per core: 25088 = 24*1024 + 512
NPAD = PER * NCORES    # 200704
SUPER = 1024           # nodes per superblock (8 subtiles of 128)
NSUPER = 25            # 24 full superblocks + 1 half (512 nodes)
YCOLS = PER // 128     # 196

BF16 = ml_dtypes.bfloat16

_PROGS = {}
VARIANT = "hostT2"  # hostT2 | hostT | hostperm | inter2 | base | plainload | inter


def _build_program(reps=1):
    import contextlib

    import concourse.tile as tile
    from concourse import bacc, mybir

    BF = mybir.dt.bfloat16
    F32 = mybir.dt.float32
    AF = mybir.ActivationFunctionType
    OP = mybir.AluOpType

    nc = bacc.Bacc("TRN2", target_bir_lowering=False, debug=False,
                   num_devices=NCORES)

    if VARIANT == "hostT2":
        # host feeds per-superblock transposed contiguous blocks (c, p, j)
        x_d = nc.dram_tensor("x", [2 * PER * 128], BF, kind="ExternalInput").ap()
    elif VARIANT == "hostT":
        # host feeds x already transposed: row f = feature, col = node
        x_d = nc.dram_tensor("x", [F, PER], BF, kind="ExternalInput").ap()
    elif VARIANT == "hostperm":
        # host pre-permutes x so each (superblock, chunk) transpose source
        # is one contiguous [nn, 128] block
        x_d = nc.dram_tensor("x", [2 * PER, 128], BF, kind="ExternalInput").ap()
    else:
        x_d = nc.dram_tensor("x", [PER, F], BF, kind="ExternalInput").ap()
    acat_d = nc.dram_tensor("acat", [2, 128, 64], BF, kind="ExternalInput").ap()
    bias_d = nc.dram_tensor("biascat", [1, 512], BF, kind="ExternalInput").ap()
    wl_d = nc.dram_tensor("wlfull", [128, 256], BF, kind="ExternalInput").ap()
    ones_d = nc.dram_tensor("ones", [1, 128], BF, kind="ExternalInput").ap()
    id_d = nc.dram_tensor("ident", [128, 128], F32, kind="ExternalInput").ap()
    y_d = nc.dram_tensor("y", [YCOLS, 128], F32, kind="ExternalOutput").ap()

    with tile.TileContext(nc) as tc:
        with tc.tile_pool(name="const", bufs=1) as cp, \
             tc.tile_pool(name="xt", bufs=8) as xp, \
             tc.tile_pool(name="act", bufs=6) as vp, \
             tc.tile_pool(name="ps", bufs=6, space="PSUM") as pp, \
             tc.tile_pool(name="yps", bufs=2, space="PSUM") as yp:

            acat0 = cp.tile([128, 64], BF)
            acat1 = cp.tile([128, 64], BF)
            biascat = cp.tile([1, 512], BF)
            wlfull = cp.tile([128, 256], BF)
            ones = cp.tile([1, 128], BF)
            ident = cp.tile([128, 128], F32)
            ysb = cp.tile([128, YCOLS], F32)

            nc.scalar.dma_start(out=acat0[:], in_=acat_d[0])
            nc.scalar.dma_start(out=acat1[:], in_=acat_d[1])
            nc.scalar.dma_start(out=biascat[:], in_=bias_d[:])
            nc.scalar.dma_start(out=wlfull[:], in_=wl_d[:])
            nc.scalar.dma_start(out=ones[:], in_=ones_d[:])
            nc.scalar.dma_start(out=ident[:], in_=id_d[:])

            rep_ctx = (tc.For_i(0, reps, 1,
                               hint_engines=(mybir.EngineType.PE,
                                             mybir.EngineType.SP))
                       if reps > 1 else contextlib.nullcontext())
            with rep_ctx:
                _kernel_body(nc, tc, mybir, BF, F32, AF, OP,
                             x_d, y_d, xp, vp, pp, yp,
                             acat0, acat1, biascat, wlfull, ones, ident, ysb)

    nc.compile()
    return nc


def _kernel_body(nc, tc, mybir, BF, F32, AF, OP, x_d, y_d, xp, vp, pp, yp,
                 acat0, acat1, biascat, wlfull, ones, ident, ysb):
    if True:
        if True:
            for b in range(NSUPER):
                nsub = 8 if b < NSUPER - 1 else 4
                nn = nsub * 128
                base = b * SUPER

                if VARIANT == "hostT2":
                    xt = xp.tile([128, 2 * SUPER], BF, tag="xt")
                    off = base * 256
                    eng = (nc.sync, nc.scalar, nc.gpsimd)[b % 3]
                    eng.dma_start(
                        out=xt[:, :2 * nn].rearrange("p (c j) -> p c j", c=2),
                        in_=x_d[off:off + 256 * nn].rearrange(
                            "(c p j) -> p c j", c=2, p=128))

                    def _lhs(s, c, xt=xt, nn=nn):
                        return xt[:, c * nn + s * 128:c * nn + (s + 1) * 128]
                elif VARIANT == "hostT":
                    xt0t = xp.tile([128, SUPER], BF, tag="xt0")
                    xt1t = xp.tile([128, SUPER], BF, tag="xt1")
                    nc.sync.dma_start(out=xt0t[:, :nn],
                                      in_=x_d[0:128, base:base + nn])
                    nc.sync.dma_start(out=xt1t[:, :nn],
                                      in_=x_d[128:256, base:base + nn])

                    def _lhs(s, c, xt0=xt0t, xt1=xt1t):
                        t = xt0 if c == 0 else xt1
                        return t[:, s * 128:(s + 1) * 128]
                elif VARIANT == "hostperm":
                    xt0t = xp.tile([128, SUPER], BF, tag="xt0")
                    xt1t = xp.tile([128, SUPER], BF, tag="xt1")
                    r0 = 2 * base
                    nc.sync.dma_start(out=xt0t[:, :nn],
                                      in_=x_d[r0:r0 + nn, :],
                                      transpose=True)
                    nc.sync.dma_start(out=xt1t[:, :nn],
                                      in_=x_d[r0 + nn:r0 + 2 * nn, :],
                                      transpose=True)

                    def _lhs(s, c, xt0=xt0t, xt1=xt1t):
                        t = xt0 if c == 0 else xt1
                        return t[:, s * 128:(s + 1) * 128]
                elif VARIANT in ("inter2", "nocompute"):
                    # two contiguous-source transposes; columns interleave
                    # (node, chunk) pairs
                    x2 = x_d.rearrange("n (a c) -> (n a) c", c=128)
                    tA = xp.tile([128, SUPER], BF, tag="xtA")
                    tB = xp.tile([128, SUPER], BF, tag="xtB")
                    nc.sync.dma_start(out=tA[:, :nn],
                                      in_=x2[2 * base:2 * base + nn, :],
                                      transpose=True)
                    nc.sync.dma_start(out=tB[:, :nn],
                                      in_=x2[2 * base + nn:2 * base + 2 * nn, :],
                                      transpose=True)
                    tA3 = tA[:, :nn].rearrange("p (j c) -> p c j", c=2)
                    tB3 = tB[:, :nn].rearrange("p (j c) -> p c j", c=2)
                    half_sub = nsub // 2

                    def _lhs(s, c, tA3=tA3, tB3=tB3, half_sub=half_sub):
                        t3 = tA3 if s < half_sub else tB3
                        j0 = (s % half_sub) * 128
                        return t3[:, c, j0:j0 + 128]
                elif VARIANT == "inter":
                    # single contiguous-source transpose; even columns are
                    # feature chunk 0, odd columns chunk 1
                    x2 = x_d.rearrange("n (a c) -> (n a) c", c=128)
                    xti = xp.tile([128, 2 * SUPER], BF, tag="xti")
                    nc.sync.dma_start(out=xti[:, :2 * nn],
                                      in_=x2[2 * base:2 * (base + nn), :],
                                      transpose=True)
                    xt3 = xti[:, :2 * nn].rearrange("p (j c) -> p c j", c=2)

                    def _lhs(s, c, xt3=xt3):
                        return xt3[:, c, s * 128:(s + 1) * 128]
                elif VARIANT == "plainload":
                    # timing probe only: same bytes, no transpose (wrong data)
                    xt0 = xp.tile([128, SUPER], BF, tag="xt0")
                    xt1 = xp.tile([128, SUPER], BF, tag="xt1")
                    xv = x_d[base:base + nn, :].rearrange(
                        "(p a) f -> p (a f)", p=128)
                    nc.sync.dma_start(out=xt0[:, :nn], in_=xv[:, :nn])
                    nc.sync.dma_start(out=xt1[:, :nn], in_=xv[:, nn:2 * nn])

                    def _lhs(s, c, xt0=xt0, xt1=xt1):
                        t = xt0 if c == 0 else xt1
                        return t[:, s * 128:(s + 1) * 128]
                else:
                    xt0t = xp.tile([128, SUPER], BF, tag="xt0")
                    xt1t = xp.tile([128, SUPER], BF, tag="xt1")
                    nc.sync.dma_start(out=xt0t[:, :nn],
                                      in_=x_d[base:base + nn, 0:128],
                                      transpose=True)
                    nc.sync.dma_start(out=xt1t[:, :nn],
                                      in_=x_d[base:base + nn, 128:256],
                                      transpose=True)

                    def _lhs(s, c, xt0=xt0t, xt1=xt1t):
                        t = xt0 if c == 0 else xt1
                        return t[:, s * 128:(s + 1) * 128]

                if VARIANT == "nocompute":
                    # timing probe: force DMA completion with tiny reads
                    nc.vector.tensor_copy(ysb[:, b:b + 1], _lhs(0, 0)[:, 0:1])
                    nc.vector.tensor_copy(ysb[:, b:b + 1], _lhs(0, 1)[:, 0:1])
                    continue

                ps = pp.tile([128, 512], F32, tag="ps")
                # rank-1 bias broadcast fills the bank and opens the group
                nc.tensor.matmul(ps[:, :nsub * 64], ones[:],
                                 biascat[:, :nsub * 64],
                                 start=True, stop=False)
                for s in range(nsub):
                    out_sl = ps[:, s * 64:(s + 1) * 64]
                    nc.tensor.matmul(out_sl, _lhs(s, 0), acat0[:],
                                     start=False, stop=False)
                    nc.tensor.matmul(out_sl, _lhs(s, 1), acat1[:],
                                     start=False, stop=(s == nsub - 1))

                ps3 = ps[:, :nsub * 64].rearrange("p (s h) -> p s h", h=64)
                zc = vp.tile([128, 256], BF, tag="zc")
                tt = vp.tile([128, 256], BF, tag="tt")
                zc3 = zc[:, :nsub * 32].rearrange("p (s h) -> p s h", h=32)
                tt3 = tt[:, :nsub * 32].rearrange("p (s h) -> p s h", h=32)
                nc.scalar.activation(zc3, ps3[:, :, 0:32], AF.Sigmoid,
                                     scale=-1.0)
                nc.scalar.activation(tt3, ps3[:, :, 32:64], AF.Tanh)

                gr = vp.tile([128, 256], BF, tag="gr")
                gw = vp.tile([128, 256], BF, tag="gw")
                # gr = relu(tt) * zc  (zc > 0 so this equals relu(zc*tt))
                nc.vector.scalar_tensor_tensor(
                    gr[:, :nsub * 32], tt[:, :nsub * 32], 0.0,
                    zc[:, :nsub * 32], op0=OP.max, op1=OP.mult)
                nc.vector.tensor_mul(gw[:, :nsub * 32], gr[:, :nsub * 32],
                                     wlfull[:, :nsub * 32])
                gw3 = gw[:, :nsub * 32].rearrange("p (s h) -> p s h", h=32)
                nc.vector.tensor_reduce(ysb[:, b * 8:b * 8 + nsub], gw3,
                                        axis=mybir.AxisListType.X, op=OP.add)

                # flush finished halves of ysb mid-loop to shorten the tail
                if b == 11 or b == NSUPER - 1:
                    h0 = 0 if b == 11 else 96
                    hw = 96 if b == 11 else YCOLS - 96  # 96 then 100
                    ytp = yp.tile([128, 128], F32, tag="ytp")
                    nc.tensor.transpose(ytp[:hw, :],
                                        ysb[:, h0:h0 + hw], ident[:])
                    yts = vp.tile([128, 128], F32, tag="yts")
                    nc.vector.tensor_copy(yts[:hw, :], ytp[:hw, :])
                    nc.sync.dma_start(out=y_d[h0:h0 + hw, :],
                                      in_=yts[:hw, :])


def _get_program(reps=1):
    if reps not in _PROGS:
        _PROGS[reps] = _build_program(reps)
    return _PROGS[reps]


def _host_inputs(x, Wz, bz, Wh, bh, Wl):
    Az = (np.asarray(Wz[0]) + np.asarray(Wz[1]))[:F]
    Ah = (np.asarray(Wh[0]) + np.asarray(Wh[1]))[:F]
    Acat = np.concatenate([Az, Ah], axis=1)               # [256, 64]
    acat = np.stack([Acat[:128], Acat[128:]]).astype(BF16)
    biascat = np.concatenate([np.asarray(bz), np.asarray(bh)])  # [64]
    biascat8 = np.tile(biascat, 8)[None, :].astype(BF16)  # [1, 512]
    wlfull = np.tile(np.asarray(Wl).reshape(1, HID), (128, 8)).astype(BF16)
    ones = np.ones((1, 128), BF16)
    ident = np.eye(128, dtype=np.float32)

    xb = np.zeros((NPAD, F), dtype=BF16)
    xb[:N] = np.asarray(x).astype(BF16)
    shards = xb.reshape(NCORES, PER, F)
    if VARIANT == "hostT2":
        nfull = (NSUPER - 1) * SUPER
        main = shards[:, :nfull].reshape(NCORES, NSUPER - 1, SUPER, F)
        main = main.transpose(0, 1, 3, 2).reshape(NCORES, -1)
        tail = shards[:, nfull:].reshape(NCORES, 1, PER - nfull, F)
        tail = tail.transpose(0, 1, 3, 2).reshape(NCORES, -1)
        shards = np.concatenate([main, tail], axis=1)  # [NCORES, 2*PER*128]
    elif VARIANT == "hostT":
        # [NCORES, PER, F] -> [NCORES, F, PER]
        shards = np.ascontiguousarray(shards.transpose(0, 2, 1))
    elif VARIANT == "hostperm":
        # [(b sup) (c f)] -> [(b c sup) f]: every (superblock, chunk)
        # transpose source becomes one contiguous [sup, 128] block
        nfull = (NSUPER - 1) * SUPER  # 24576
        main = shards[:, :nfull].reshape(NCORES, NSUPER - 1, SUPER, 2, 128)
        main = main.transpose(0, 1, 3, 2, 4).reshape(NCORES, -1, 128)
        tail = shards[:, nfull:].reshape(NCORES, 1, PER - nfull, 2, 128)
        tail = tail.transpose(0, 1, 3, 2, 4).reshape(NCORES, -1, 128)
        shards = np.concatenate([main, tail], axis=1)  # [NCORES, 2*PER, 128]
    return shards, acat, biascat8, wlfull, ones, ident


def kernel(x, edge_index, Wz, bz, Wr, br, Wh, bh, Wl, bl, _reps=1):
    from concourse.bass_utils import run_bass_kernel_spmd

    shards, acat, biascat8, wlfull, ones, ident = _host_inputs(
        x, Wz, bz, Wh, bh, Wl)

    nc = _get_program(_reps)
    in_maps = [{
        "x": np.ascontiguousarray(shards[i]),
        "acat": acat,
        "biascat": biascat8,
        "wlfull": wlfull,
        "ones": ones,
        "ident": ident,
    } for i in range(NCORES)]

    res = run_bass_kernel_spmd(nc, in_maps, core_ids=list(range(NCORES)))

    y = np.concatenate([np.asarray(res.results[i]["y"]).reshape(-1)
                        for i in range(NCORES)])[:N]
    out = (y + np.float32(np.asarray(bl).reshape(-1)[0])).astype(np.float32)
    return out.reshape(N, 1)
